# revision 9
# baseline (speedup 1.0000x reference)
"""Trainium2 Bass kernel for windowed (sparse) attention — v4.

Module: LayerNorm -> overlapping 8x8 spatial windows (stride 6) over a
[2,2,128,128,256] image -> per-window 8-head attention over L=128 tokens
(t*8*8) -> output projection -> overlap-add with count normalization.

Strategy: 882 independent windows sharded over 8 cores (112 each, padded
to 896).  Host does im2win gather + overlap-add scatter; all model
compute runs on device.

Performance history: v1 1817us -> v3 337us -> v4 (this).
v4 changes vs v3 (trace-driven: Scalar 93.8%, Vector 86.2%, GpSimd
qz-build 2.1us/window were the bottlenecks; PE only ~71%):
  - qz (zero-padded head-major q for the K=128 S matmuls) is built by a
    band-scatter SBUF->SBUF DMA instead of a gpsimd masked multiply:
    each q element lands in exactly one band of the padded layout, the
    zero regions are memset once at startup and never rewritten.
  - LN apply moved from DVE (tensor_scalar, 711ns/w) to GpSimd, which
    has no other work left.
  - Most stages process a PAIR of windows per instruction (halves
    instruction overheads and semaphore traffic).  PSUM: tp(1) qkp(2)
    vp(1) sp(2) dp+op merged(1) zp(1) = 8 banks.
  - Evac balance: qks evac on ACT; xnt/vs/zs evacs + recip + normalize
    on DVE; es Exp on ACT (forced single activation-table set).
"""

import functools
import math
from contextlib import ExitStack

import numpy as np

import concourse.bacc as bacc
import concourse.bass as bass
import concourse.mybir as mybir
import concourse.tile as tile
from concourse.bass import AP
from concourse.bass_utils import run_bass_kernel_spmd

# Force every ACT function this kernel uses (Exp, Ln, Copy) into the single
# combined table set `natural_log_exp_and_others` so the scalar engine never
# swaps activation tables (each swap costs ~2.7us).
_ORIG_GAT = bacc.get_activation_tables

def _patched_gat(arch):
    tabs = _ORIG_GAT(arch)
    strip = {mybir.ActivationFunctionType.from_pwp(n)
             for n in ("exp", "ln", "copy", "identity")}
    return {n: (fns if n == "natural_log_exp_and_others" else fns - strip)
            for n, fns in tabs.items()}

bacc.get_activation_tables = _patched_gat

# Problem constants (hardcoded per contract - kernel.py is self-contained).
B, T, H, W, C = 2, 2, 128, 128, 256
MID, HEADS = 256, 8
HD = MID // HEADS          # 32
PATCH, STEP = 8, 6         # window size / stride
NHW = 21                   # windows per axis: starts 0,6,...,120
NWIN = NHW * NHW * B       # 882 flat windows (n outer, b inner)
L = T * PATCH * PATCH      # 128 tokens per window
NCORES = 8
NW = 112                   # windows per core after padding to 896
EPS = 1e-6
F32, F16 = mybir.dt.float32, mybir.dt.float16
AF = mybir.ActivationFunctionType
ALU = mybir.AluOpType


def _build_program(nw: int):
    nc = bacc.Bacc(
        "TRN2",
        target_bir_lowering=False,
        debug=False,
        enable_asserts=False,
        num_devices=NCORES,
    )
    xw = nc.dram_tensor("xw", [nw * 128, 256], F16, kind="ExternalInput").ap()
    wq = nc.dram_tensor("wq", [256, 256], F16, kind="ExternalInput").ap()
    wk = nc.dram_tensor("wk", [256, 256], F16, kind="ExternalInput").ap()
    wv = nc.dram_tensor("wv", [256, 256], F16, kind="ExternalInput").ap()
    wo = nc.dram_tensor("wo", [256, 256], F16, kind="ExternalInput").ap()
    ones1 = nc.dram_tensor("ones1", [128, 32], F16, kind="ExternalInput").ap()
    ident = nc.dram_tensor("ident", [128, 128], F16, kind="ExternalInput").ap()
    zt = nc.dram_tensor("zt", [nw * 256, 128], F16, kind="ExternalOutput").ap()

    inv_sqrt_hd = 1.0 / math.sqrt(HD)
    assert nw % 4 == 0
    GROUPS = nw // 4
    NP = nw // 2  # pairs

    with tile.TileContext(nc) as tc, ExitStack() as ctx:
        pw = ctx.enter_context(tc.tile_pool(name="wts", bufs=1))
        wq_s = [pw.tile([128, 256], F16, tag=f"wq{i}", name=f"wq{i}") for i in range(2)]
        wk_s = [pw.tile([128, 256], F16, tag=f"wk{i}", name=f"wk{i}") for i in range(2)]
        wv_s = [pw.tile([128, 256], F16, tag=f"wv{i}", name=f"wv{i}") for i in range(2)]
        wo_s = [pw.tile([128, 256], F16, tag=f"wo{i}", name=f"wo{i}") for i in range(2)]
        for i in range(2):
            nc.sync.dma_start(wq_s[i][:], wq[i * 128:(i + 1) * 128, :])
            nc.sync.dma_start(wk_s[i][:], wk[i * 128:(i + 1) * 128, :])
            nc.sync.dma_start(wv_s[i][:], wv[i * 128:(i + 1) * 128, :])
            nc.sync.dma_start(wo_s[i][:], wo[i * 128:(i + 1) * 128, :])
        ones_s = pw.tile([128, 32], F16, tag="ones1", name="ones1")
        nc.sync.dma_start(ones_s[:], ones1)
        id_s = pw.tile([128, 128], F16, tag="ident", name="ident")
        nc.sync.dma_start(id_s[:], ident)
        eps_s = pw.tile([128, 1], F32, tag="eps", name="eps")
        nc.vector.memset(eps_s[:], EPS)

        # qz tiles: 3 fixed buffers, zero regions written once here and
        # never again (the per-pair DMA only rewrites the q bands).
        qz_t = [pw.tile([128, 2048], F16, tag=f"qz{i}", name=f"qz{i}")
                for i in range(3)]
        for t_ in qz_t:
            nc.gpsimd.memset(t_[:], 0.0)

        # SBUF pools (pair-granular tiles unless noted)
        pxt = ctx.enter_context(tc.tile_pool(name="pxt", bufs=4))
        pst = ctx.enter_context(tc.tile_pool(name="pst", bufs=4))
        pxn = ctx.enter_context(tc.tile_pool(name="pxn", bufs=3))
        pxnt = ctx.enter_context(tc.tile_pool(name="pxnt", bufs=3))
        pqks = ctx.enter_context(tc.tile_pool(name="pqks", bufs=3))
        pvs = ctx.enter_context(tc.tile_pool(name="pvs", bufs=4))
        pes = ctx.enter_context(tc.tile_pool(name="pes", bufs=3))
        pdb = ctx.enter_context(tc.tile_pool(name="pdb", bufs=3))
        pos = ctx.enter_context(tc.tile_pool(name="pos", bufs=3))
        pzs = ctx.enter_context(tc.tile_pool(name="pzs", bufs=3))
        # PSUM pools: 1 + 2 + 1 + 2 + 1 + 1 = 8 banks
        ptp = ctx.enter_context(tc.tile_pool(name="ptp", bufs=1, space="PSUM"))
        pqk = ctx.enter_context(tc.tile_pool(name="pqk", bufs=1, space="PSUM"))
        pv = ctx.enter_context(tc.tile_pool(name="pv", bufs=1, space="PSUM"))
        psp = ctx.enter_context(tc.tile_pool(name="psp", bufs=1, space="PSUM"))
        pd = ctx.enter_context(tc.tile_pool(name="pd", bufs=1, space="PSUM"))
        pz = ctx.enter_context(tc.tile_pool(name="pz", bufs=1, space="PSUM"))

        # Pipeline state
        xt_g = {}
        bag_p, rs_p = {}, {}
        xn_p, xnt_p, qks_p, vs_p, os_p, zs_p = {}, {}, {}, {}, {}, {}
        es_w, dop_w = {}, {}
        zp_p = {}

        def load_group(g):
            t_ = pxt.tile([128, 1024], F16, tag="xt", name="xt")
            src = AP(xw.tensor, g * 4 * 128 * 256,
                     [[256, 128], [128 * 256, 4], [1, 256]])
            nc.sync.dma_start(t_[:], src)
            xt_g[g] = t_
            xt_g.pop(g - 3, None)

        load_group(0)

        for i in range(nw + 17):
            if (i + 2) % 4 == 0:
                g = (i + 2) // 4
                if g < GROUPS:
                    load_group(g)

            # ---- k=0 (pair): LN stats + rsqrt ----
            w = i
            if 0 <= w < nw and w % 2 == 0:
                p = w // 2
                g, h = p // 2, p % 2  # group, half-of-group
                xt = xt_g[g]
                bag = pst.tile([128, 4], F32, tag="bag", name="bag")
                for w_ in range(2):
                    bst = pst.tile([128, 6], F32, tag="bst", name="bst")
                    nc.vector.bn_stats(
                        bst[:],
                        xt[:, h * 512 + w_ * 256: h * 512 + (w_ + 1) * 256])
                    nc.vector.bn_aggr(bag[:, 2 * w_:2 * w_ + 2], bst[:])
                var2 = AP(bag[:].tensor, bag[:].offset + 1, [[4, 128], [2, 2]])
                lnv = pst.tile([128, 2], F32, tag="lnv", name="lnv")
                nc.scalar.activation(lnv[:], var2, AF.Ln, bias=eps_s[:])
                rs2 = pst.tile([128, 2], F32, tag="rs2", name="rs2")
                nc.scalar.activation(rs2[:], lnv[:], AF.Exp, scale=-0.5)
                bag_p[p], rs_p[p] = bag, rs2

            # ---- k=2 (pair): LN apply on gpsimd -> xn f16 [128,512] ----
            w = i - 2
            if 0 <= w < nw and w % 2 == 0:
                p = w // 2
                g, h = p // 2, p % 2
                xt = xt_g[g]
                bag, rs2 = bag_p[p], rs_p[p]
                xn = pxn.tile([128, 512], F16, tag="xn", name="xn")
                for w_ in range(2):
                    nc.gpsimd.tensor_scalar(
                        out=xn[:, w_ * 256:(w_ + 1) * 256],
                        in0=xt[:, h * 512 + w_ * 256: h * 512 + (w_ + 1) * 256],
                        scalar1=bag[:, 2 * w_:2 * w_ + 1],
                        scalar2=rs2[:, w_:w_ + 1],
                        op0=ALU.subtract, op1=ALU.mult,
                    )
                xn_p[p] = xn

            # ---- k=4 (pair): PE transpose (4 MMs) ----
            w = i - 4
            if 0 <= w < nw and w % 2 == 0:
                p = w // 2
                tp = ptp.tile([128, 512], F16, tag="tp", name="tp")
                xn = xn_p[p]
                for c4 in range(4):
                    nc.tensor.transpose(
                        tp[:, c4 * 128:(c4 + 1) * 128],
                        xn[:, c4 * 128:(c4 + 1) * 128], id_s[:])
                zp_p[("tp", p)] = tp
                del xn_p[p]

            # ---- k=5 (pair): xnt evac (DVE, f16 2x) ----
            w = i - 5
            if 0 <= w < nw and w % 2 == 0:
                p = w // 2
                tp = zp_p.pop(("tp", p))
                xnt = pxnt.tile([128, 512], F16, tag="xnt", name="xnt")
                nc.vector.tensor_copy(xnt[:], tp[:])
                xnt_p[p] = xnt

            # ---- k=6 (pair): q/k/v projections ----
            w = i - 6
            if 0 <= w < nw and w % 2 == 0:
                p = w // 2
                xnt = xnt_p[p]
                qkp = pqk.tile([128, 1024], F32, tag="qk", name="qk")
                # layout: cols qk*512 + mh*256 + w'*128 + l (each MM output
                # is a contiguous 256-col chunk within one PSUM bank)
                for qk_i, ws in ((0, wq_s), (1, wk_s)):
                    for mh in range(2):
                        for kc in range(2):
                            rhs = AP(xnt[:].tensor, xnt[:].offset + kc * 128,
                                     [[512, 128], [256, 2], [1, 128]])
                            nc.tensor.matmul(
                                qkp[:, qk_i * 512 + mh * 256: qk_i * 512 + (mh + 1) * 256],
                                lhsT=ws[kc][:, mh * 128:(mh + 1) * 128],
                                rhs=rhs,
                                start=(kc == 0), stop=(kc == 1),
                                skip_group_check=True,
                            )
                vp = pv.tile([128, 512], F32, tag="v", name="v")
                for w_ in range(2):
                    for kc in range(2):
                        nc.tensor.matmul(
                            vp[:, w_ * 256:(w_ + 1) * 256],
                            lhsT=xnt[:, w_ * 256 + kc * 128: w_ * 256 + (kc + 1) * 128],
                            rhs=wv_s[kc][:],
                            start=(kc == 0), stop=(kc == 1),
                            skip_group_check=True,
                        )
                zp_p[("qkp", p)] = qkp
                zp_p[("vp", p)] = vp
                del xnt_p[p]

            # ---- k=7 (pair): qks evac (ACT) + vs evac (DVE) ----
            w = i - 7
            if 0 <= w < nw and w % 2 == 0:
                p = w // 2
                qkp = zp_p.pop(("qkp", p))
                vp = zp_p.pop(("vp", p))
                qks = pqks.tile([128, 1024], F16, tag="qks", name="qks")
                nc.scalar.copy(qks[:], qkp[:])
                vs = pvs.tile([128, 512], F16, tag="vs", name="vs")
                nc.vector.tensor_copy(vs[:], vp[:])
                qks_p[p], vs_p[p] = qks, vs

            # ---- k=8 (pair): qz band-scatter DMA (SBUF->SBUF) ----
            w = i - 8
            if 0 <= w < nw and w % 2 == 0:
                p = w // 2
                qks = qks_p[p]
                qz = qz_t[p % 3]
                # Band j is a contiguous block copy: qz[32j:32j+32,
                # j*512:(j+1)*512] <- qks[32j:32j+32, 0:512] (the q half).
                # Zeros outside the bands were memset once at startup; the
                # S matmul rhs AP interleaves (j, l) columns at read time.
                for j in range(4):
                    nc.sync.dma_start(
                        qz[32 * j:32 * j + 32, j * 512:(j + 1) * 512],
                        qks[32 * j:32 * j + 32, 0:512])
                zp_p[("qz", p)] = qz

            # ---- k=9 (window): S matmuls (2x N=512, K=128 padded) ----
            w = i - 9
            if 0 <= w < nw:
                p, w_ = w // 2, w % 2
                qks = qks_p[w // 2]
                qz = zp_p[("qz", p)]
                sp = psp.tile([128, 1024], F32, tag="sp", name="sp")
                for mh in range(2):
                    rhs = AP(qz[:].tensor,
                             qz[:].offset + mh * 256 + w_ * 128,
                             [[2048, 128], [512, 4], [1, 128]])
                    nc.tensor.matmul(
                        sp[:, mh * 512:(mh + 1) * 512],
                        lhsT=qks[:, 512 + mh * 256 + w_ * 128: 512 + mh * 256 + w_ * 128 + 128],
                        rhs=rhs,
                        start=True, stop=True,
                    )
                zp_p[("sp", w)] = sp
                if w_ == 1:
                    zp_p.pop(("qz", p), None)

            # ---- k=10 (window): es = Exp(sp * scale) on ACT ----
            w = i - 10
            if 0 <= w < nw:
                sp = zp_p.pop(("sp", w))
                es = pes.tile([128, 1024], F16, tag="es", name="es")
                nc.scalar.activation(es[:], sp[:], AF.Exp, scale=inv_sqrt_hd)
                es_w[w] = es

            # ---- k=11 (window): D (4x N=256) + OT (8x) into merged bank ----
            w = i - 11
            if 0 <= w < nw:
                p, w_ = w // 2, w % 2
                es, vs = es_w[w], vs_p[p]
                dop = pd.tile([128, 512], F32, tag="dop", name="dop")
                for j in range(4):
                    rhs = AP(es[:].tensor, es[:].offset + j * 128,
                             [[1024, 128], [512, 2], [1, 128]])
                    nc.tensor.matmul(
                        dop[32 * j:32 * j + 32, 0:256],
                        lhsT=ones_s[:], rhs=rhs,
                        start=True, stop=True, tile_position=(0, 32 * j),
                        skip_group_check=True,
                    )
                for h in range(HEADS):
                    r, j = h // 4, h % 4
                    nc.tensor.matmul(
                        dop[32 * j:32 * j + 32, 256 + r * 128:256 + (r + 1) * 128],
                        lhsT=vs[:, w_ * 256 + 32 * h: w_ * 256 + 32 * h + 32],
                        rhs=es[:, h * 128:(h + 1) * 128],
                        start=(r == 0), stop=(r == 1), tile_position=(0, 32 * j),
                        skip_group_check=True,
                    )
                dop_w[w] = dop
                del es_w[w]

            # ---- k=12 (window): softmax normalize on DVE ----
            w = i - 12
            if 0 <= w < nw:
                p, w_ = w // 2, w % 2
                dop = dop_w.pop(w)
                dbs = pdb.tile([128, 256], F32, tag="dbs", name="dbs")
                nc.vector.reciprocal_approx_fast(out=dbs[:], in_=dop[:, 0:256])
                if w_ == 0:
                    os_p[p] = pos.tile([128, 512], F16, tag="os", name="os")
                os_ = os_p[p]
                nc.vector.scalar_tensor_tensor(
                    out=os_[:, w_ * 256:(w_ + 1) * 256],
                    in0=dop[:, 256:512], scalar=1.0, in1=dbs[:],
                    op0=ALU.mult, op1=ALU.mult,
                )

            # ---- k=13 (pair): out projection (4x N=256) ----
            w = i - 13
            if 0 <= w < nw and w % 2 == 0:
                p = w // 2
                os_ = os_p[p]
                zp = pz.tile([128, 512], F32, tag="zp", name="zp")
                # cols: w'*256 + coh*128 + l
                for coh in range(2):
                    for kc in range(2):
                        rhs = AP(os_[:].tensor, os_[:].offset + kc * 128,
                                 [[512, 128], [256, 2], [1, 128]])
                        out = AP(zp[:].tensor, zp[:].offset + coh * 128,
                                 [[512, 128], [256, 2], [1, 128]])
                        nc.tensor.matmul(
                            out,
                            lhsT=wo_s[kc][:, coh * 128:(coh + 1) * 128],
                            rhs=rhs,
                            start=(kc == 0), stop=(kc == 1),
                            skip_group_check=True,
                        )
                zp_p[("zp", p)] = zp
                del os_p[p]

            # ---- k=15 (pair): zs evac (DVE) ----
            w = i - 15
            if 0 <= w < nw and w % 2 == 0:
                p = w // 2
                zp = zp_p.pop(("zp", p))
                zs = pzs.tile([128, 512], F16, tag="zs", name="zs")
                nc.vector.tensor_copy(zs[:], zp[:])
                zs_p[p] = zs
                del qks_p[p], vs_p[p]

            # ---- k=16 (pair): store DMA ----
            w = i - 16
            if 0 <= w < nw and w % 2 == 0:
                p = w // 2
                zs = zs_p.pop(p)
                # zs cols: w'*256 + coh*128 + l ; zt rows (2p+w')*256+coh*128+c
                dst = AP(zt.tensor, (2 * p) * 256 * 128,
                         [[128, 128], [256 * 128, 2], [128 * 128, 2], [1, 128]])
                src = AP(zs[:].tensor, zs[:].offset,
                         [[512, 128], [256, 2], [128, 2], [1, 128]])
                nc.sync.dma_start(dst, src)
    nc.compile()
    return nc


@functools.lru_cache(maxsize=2)
def _get_program(nw: int):
    return _build_program(nw)


def _im2win(x: np.ndarray) -> np.ndarray:
    """[B,T,H,W,C] -> [882,128,256] windows, flat order f = i_n*B + i_b."""
    s = x.strides
    xs = np.lib.stride_tricks.as_strided(
        x,
        shape=(B, T, NHW, PATCH, NHW, PATCH, C),
        strides=(s[0], s[1], STEP * s[2], s[2], STEP * s[3], s[3], s[4]),
    )
    w = xs.transpose(2, 4, 0, 1, 3, 5, 6)  # [iH,iW,b,t,p,q,c]
    return np.ascontiguousarray(w.reshape(NHW * NHW * B, L, C))


def _overlap_add(zwin: np.ndarray, bo: np.ndarray) -> np.ndarray:
    """[882,128,256] window outputs -> [B,T,H,W,C] with count-normalize + bo."""
    th = np.arange(NHW) * STEP
    z = zwin.reshape(B, NHW, NHW, T, PATCH, PATCH, MID)  # [b,iH,iW,t,p,q,c]
    acc = np.zeros((B, T, H, W, MID), np.float32)
    count = np.zeros((H, W), np.float32)
    for p in range(PATCH):
        rid = (th + p)[:, None]
        for q in range(PATCH):
            cid = (th + q)[None, :]
            acc[:, :, rid, cid, :] += z[:, :, :, :, p, q, :].transpose(0, 3, 1, 2, 4)
            count[rid, cid] += 1.0
    out = acc / count[None, None, :, :, None] + bo[None, None, None, None, :]
    return out


LAST_RESULT = None


def kernel(x, ln_g, ln_b, Wq, Wk, Wv, Wo, bo):
    x = np.asarray(x, np.float32)
    ln_g = np.asarray(ln_g, np.float32)
    ln_b = np.asarray(ln_b, np.float32)
    assert np.allclose(ln_b, 0.0), "kernel folds ln_g into weights; ln_b must be 0"
    # Fold LN gamma into the input side of Wq/Wk/Wv.
    wq_t = np.ascontiguousarray((np.asarray(Wq, np.float32) * ln_g).T.astype(np.float16))
    wk_t = np.ascontiguousarray((np.asarray(Wk, np.float32) * ln_g).T.astype(np.float16))
    wv_t = np.ascontiguousarray((np.asarray(Wv, np.float32) * ln_g).T.astype(np.float16))
    wo_t = np.ascontiguousarray(np.asarray(Wo, np.float32).T.astype(np.float16))
    ones1 = np.ones((128, 32), np.float16)
    ident = np.eye(128, dtype=np.float16)

    win = _im2win(x)                              # [882, 128, 256]
    pad = NCORES * NW - NWIN                      # 14
    winp = np.concatenate([win, np.zeros((pad, L, C), np.float32)], 0)
    shards = winp.reshape(NCORES, NW * L, C)

    nc = _get_program(NW)
    trace = bool(int(__import__("os").environ.get("KERNEL_TRACE", "0")))
    in_maps = []
    for i in range(NCORES):
        in_maps.append({
            "xw": np.ascontiguousarray(shards[i]).astype(np.float16),
            "wq": wq_t, "wk": wk_t, "wv": wv_t, "wo": wo_t,
            "ones1": ones1, "ident": ident,
        })
    res = run_bass_kernel_spmd(nc, in_maps, core_ids=list(range(NCORES)),
                               trace=trace)
    global LAST_RESULT
    LAST_RESULT = res
    zts = [np.asarray(res.results[i]["zt"], np.float32).reshape(NW, 2, 128, 128)
           for i in range(NCORES)]
    # zt rows: w*256 + c_out, cols l  ->  Z_w[l, c] = zt[w, :, :, l]
    zall = np.concatenate(zts, 0)                 # [896, 2, 128, 128]
    zwin = zall.reshape(NCORES * NW, MID, L).transpose(0, 2, 1)[:NWIN]
    return _overlap_add(np.ascontiguousarray(zwin), np.asarray(bo, np.float32))


# revision 10
# speedup vs baseline: 2.6140x; 2.6140x over previous
"""Trainium2 Bass kernel for windowed (sparse) attention — v4.

Module: LayerNorm -> overlapping 8x8 spatial windows (stride 6) over a
[2,2,128,128,256] image -> per-window 8-head attention over L=128 tokens
(t*8*8) -> output projection -> overlap-add with count normalization.

Strategy: 882 independent windows sharded over 8 cores (112 each, padded
to 896).  Host does im2win gather + overlap-add scatter; all model
compute runs on device.

Performance history: v1 1817us -> v3 337us -> v4 (this).
v4 changes vs v3 (trace-driven: Scalar 93.8%, Vector 86.2%, GpSimd
qz-build 2.1us/window were the bottlenecks; PE only ~71%):
  - qz (zero-padded head-major q for the K=128 S matmuls) is built by a
    band-scatter SBUF->SBUF DMA instead of a gpsimd masked multiply:
    each q element lands in exactly one band of the padded layout, the
    zero regions are memset once at startup and never rewritten.
  - LN apply moved from DVE (tensor_scalar, 711ns/w) to GpSimd, which
    has no other work left.
  - Most stages process a PAIR of windows per instruction (halves
    instruction overheads and semaphore traffic).  PSUM: tp(1) qkp(2)
    vp(1) sp(2) dp+op merged(1) zp(1) = 8 banks.
  - Evac balance: qks evac on ACT; xnt/vs/zs evacs + recip + normalize
    on DVE; es Exp on ACT (forced single activation-table set).
"""

import functools
import math
from contextlib import ExitStack

import numpy as np

import concourse.bacc as bacc
import concourse.bass as bass
import concourse.mybir as mybir
import concourse.tile as tile
from concourse.bass import AP
from concourse.bass_utils import run_bass_kernel_spmd

# Force every ACT function this kernel uses (Exp, Ln, Copy) into the single
# combined table set `natural_log_exp_and_others` so the scalar engine never
# swaps activation tables (each swap costs ~2.7us).
_ORIG_GAT = bacc.get_activation_tables

def _patched_gat(arch):
    tabs = _ORIG_GAT(arch)
    strip = {mybir.ActivationFunctionType.from_pwp(n)
             for n in ("exp", "ln", "copy", "identity")}
    return {n: (fns if n == "natural_log_exp_and_others" else fns - strip)
            for n, fns in tabs.items()}

bacc.get_activation_tables = _patched_gat

# Problem constants (hardcoded per contract - kernel.py is self-contained).
B, T, H, W, C = 2, 2, 128, 128, 256
MID, HEADS = 256, 8
HD = MID // HEADS          # 32
PATCH, STEP = 8, 6         # window size / stride
NHW = 21                   # windows per axis: starts 0,6,...,120
NWIN = NHW * NHW * B       # 882 flat windows (n outer, b inner)
L = T * PATCH * PATCH      # 128 tokens per window
NCORES = 8
NW = 112                   # windows per core after padding to 896
EPS = 1e-6
F32, F16 = mybir.dt.float32, mybir.dt.float16
AF = mybir.ActivationFunctionType
ALU = mybir.AluOpType


def _build_program(nw: int):
    nc = bacc.Bacc(
        "TRN2",
        target_bir_lowering=False,
        debug=False,
        enable_asserts=False,
        num_devices=NCORES,
    )
    xw = nc.dram_tensor("xw", [nw * 128, 256], F16, kind="ExternalInput").ap()
    wq = nc.dram_tensor("wq", [256, 256], F16, kind="ExternalInput").ap()
    wk = nc.dram_tensor("wk", [256, 256], F16, kind="ExternalInput").ap()
    wv = nc.dram_tensor("wv", [256, 256], F16, kind="ExternalInput").ap()
    wo = nc.dram_tensor("wo", [256, 256], F16, kind="ExternalInput").ap()
    ones1 = nc.dram_tensor("ones1", [128, 32], F16, kind="ExternalInput").ap()
    ident = nc.dram_tensor("ident", [128, 128], F16, kind="ExternalInput").ap()
    zt = nc.dram_tensor("zt", [nw * 256, 128], F16, kind="ExternalOutput").ap()

    inv_sqrt_hd = 1.0 / math.sqrt(HD)
    assert nw % 4 == 0
    GROUPS = nw // 4
    NP = nw // 2  # pairs

    with tile.TileContext(nc) as tc, ExitStack() as ctx:
        pw = ctx.enter_context(tc.tile_pool(name="wts", bufs=1))
        wq_s = [pw.tile([128, 256], F16, tag=f"wq{i}", name=f"wq{i}") for i in range(2)]
        wk_s = [pw.tile([128, 256], F16, tag=f"wk{i}", name=f"wk{i}") for i in range(2)]
        wv_s = [pw.tile([128, 256], F16, tag=f"wv{i}", name=f"wv{i}") for i in range(2)]
        wo_s = [pw.tile([128, 256], F16, tag=f"wo{i}", name=f"wo{i}") for i in range(2)]
        for i in range(2):
            nc.sync.dma_start(wq_s[i][:], wq[i * 128:(i + 1) * 128, :])
            nc.sync.dma_start(wk_s[i][:], wk[i * 128:(i + 1) * 128, :])
            nc.sync.dma_start(wv_s[i][:], wv[i * 128:(i + 1) * 128, :])
            nc.sync.dma_start(wo_s[i][:], wo[i * 128:(i + 1) * 128, :])
        ones_s = pw.tile([128, 32], F16, tag="ones1", name="ones1")
        nc.sync.dma_start(ones_s[:], ones1)
        id_s = pw.tile([128, 128], F16, tag="ident", name="ident")
        nc.sync.dma_start(id_s[:], ident)
        eps_s = pw.tile([128, 1], F32, tag="eps", name="eps")
        nc.vector.memset(eps_s[:], EPS)

        # qz tiles: 3 fixed buffers, zero regions written once here and
        # never again (the per-pair DMA only rewrites the q bands).
        qz_t = [pw.tile([128, 2048], F16, tag=f"qz{i}", name=f"qz{i}")
                for i in range(3)]
        for t_ in qz_t:
            nc.gpsimd.memset(t_[:], 0.0)

        # SBUF pools (pair-granular tiles unless noted)
        pxt = ctx.enter_context(tc.tile_pool(name="pxt", bufs=4))
        pst = ctx.enter_context(tc.tile_pool(name="pst", bufs=4))
        pxn = ctx.enter_context(tc.tile_pool(name="pxn", bufs=3))
        pxnt = ctx.enter_context(tc.tile_pool(name="pxnt", bufs=3))
        pqks = ctx.enter_context(tc.tile_pool(name="pqks", bufs=3))
        pvs = ctx.enter_context(tc.tile_pool(name="pvs", bufs=4))
        pes = ctx.enter_context(tc.tile_pool(name="pes", bufs=3))
        pdb = ctx.enter_context(tc.tile_pool(name="pdb", bufs=3))
        pos = ctx.enter_context(tc.tile_pool(name="pos", bufs=3))
        pzs = ctx.enter_context(tc.tile_pool(name="pzs", bufs=3))
        # PSUM pools: 1 + 2 + 1 + 2 + 1 + 1 = 8 banks
        ptp = ctx.enter_context(tc.tile_pool(name="ptp", bufs=1, space="PSUM"))
        pqk = ctx.enter_context(tc.tile_pool(name="pqk", bufs=1, space="PSUM"))
        pv = ctx.enter_context(tc.tile_pool(name="pv", bufs=1, space="PSUM"))
        psp = ctx.enter_context(tc.tile_pool(name="psp", bufs=1, space="PSUM"))
        pd = ctx.enter_context(tc.tile_pool(name="pd", bufs=1, space="PSUM"))
        pz = ctx.enter_context(tc.tile_pool(name="pz", bufs=1, space="PSUM"))

        # Pipeline state
        xt_g = {}
        bag_g, rs_g = {}, {}
        xn_p, xnt_p, qks_p, vs_p, os_p, zs_p = {}, {}, {}, {}, {}, {}
        es_w, dop_w = {}, {}
        misc = {}

        def load_group(g):
            t_ = pxt.tile([128, 1024], F16, tag="xt", name="xt")
            src = AP(xw.tensor, g * 4 * 128 * 256,
                     [[256, 128], [128 * 256, 4], [1, 256]])
            nc.sync.dma_start(t_[:], src)
            xt_g[g] = t_
            xt_g.pop(g - 3, None)

        load_group(0)

        for i in range(nw + 19):
            if (i + 2) % 4 == 0:
                g = (i + 2) // 4
                if g < GROUPS:
                    load_group(g)

            # ---- k=0 (pair): LN stats into group bag ----
            w = i
            if 0 <= w < nw and w % 2 == 0:
                p = w // 2
                g, h = p // 2, p % 2
                xt = xt_g[g]
                if h == 0:
                    bag_g[g] = pst.tile([128, 8], F32, tag="bag", name="bag")
                bag = bag_g[g]
                for w_ in range(2):
                    bst = pst.tile([128, 6], F32, tag="bst", name="bst")
                    nc.vector.bn_stats(
                        bst[:],
                        xt[:, (2 * h + w_) * 256:(2 * h + w_ + 1) * 256])
                    nc.vector.bn_aggr(
                        bag[:, 2 * (2 * h + w_):2 * (2 * h + w_) + 2], bst[:])

            # ---- k=3 (group): rsqrt = Exp(-0.5*Ln(var+eps)), 4 windows ----
            if (i - 3) % 4 == 0 and 0 <= (i - 3) < nw:
                g = (i - 3) // 4
                bag = bag_g[g]
                var4 = AP(bag[:].tensor, bag[:].offset + 1, [[8, 128], [2, 4]])
                lnv = pst.tile([128, 4], F32, tag="lnv", name="lnv")
                nc.scalar.activation(lnv[:], var4, AF.Ln, bias=eps_s[:])
                rs4 = pst.tile([128, 4], F32, tag="rs4", name="rs4")
                nc.scalar.activation(rs4[:], lnv[:], AF.Exp, scale=-0.5)
                rs_g[g] = rs4

            # ---- k=4 (pair): LN apply on DVE -> xn f16 [128,512] ----
            w = i - 4
            if 0 <= w < nw and w % 2 == 0:
                p = w // 2
                g, h = p // 2, p % 2
                xt = xt_g[g]
                bag, rs4 = bag_g[g], rs_g[g]
                xn = pxn.tile([128, 512], F16, tag="xn", name="xn")
                for w_ in range(2):
                    q4 = 2 * h + w_
                    nc.vector.tensor_scalar(
                        out=xn[:, w_ * 256:(w_ + 1) * 256],
                        in0=xt[:, q4 * 256:(q4 + 1) * 256],
                        scalar1=bag[:, 2 * q4:2 * q4 + 1],
                        scalar2=rs4[:, q4:q4 + 1],
                        op0=ALU.subtract, op1=ALU.mult,
                    )
                xn_p[p] = xn
                if h == 1:
                    bag_g.pop(g, None)
                    rs_g.pop(g, None)

            # ---- k=6 (pair): PE transpose (4 MMs) ----
            w = i - 6
            if 0 <= w < nw and w % 2 == 0:
                p = w // 2
                tp = ptp.tile([128, 512], F16, tag="tp", name="tp")
                xn = xn_p[p]
                for c4 in range(4):
                    nc.tensor.transpose(
                        tp[:, c4 * 128:(c4 + 1) * 128],
                        xn[:, c4 * 128:(c4 + 1) * 128], id_s[:])
                misc[("tp", p)] = tp
                del xn_p[p]

            # ---- k=7 (pair): xnt evac (DVE, f16 2x) ----
            w = i - 7
            if 0 <= w < nw and w % 2 == 0:
                p = w // 2
                tp = misc.pop(("tp", p))
                xnt = pxnt.tile([128, 512], F16, tag="xnt", name="xnt")
                nc.vector.tensor_copy(xnt[:], tp[:])
                xnt_p[p] = xnt

            # ---- k=8 (pair): q/k/v projections ----
            w = i - 8
            if 0 <= w < nw and w % 2 == 0:
                p = w // 2
                xnt = xnt_p[p]
                qkp = pqk.tile([128, 1024], F32, tag="qk", name="qk")
                # layout: cols qk*512 + mh*256 + w'*128 + l
                for qk_i, ws in ((0, wq_s), (1, wk_s)):
                    for mh in range(2):
                        for kc in range(2):
                            rhs = AP(xnt[:].tensor, xnt[:].offset + kc * 128,
                                     [[512, 128], [256, 2], [1, 128]])
                            nc.tensor.matmul(
                                qkp[:, qk_i * 512 + mh * 256: qk_i * 512 + (mh + 1) * 256],
                                lhsT=ws[kc][:, mh * 128:(mh + 1) * 128],
                                rhs=rhs,
                                start=(kc == 0), stop=(kc == 1),
                                skip_group_check=True,
                            )
                vp = pv.tile([128, 512], F32, tag="v", name="v")
                for w_ in range(2):
                    for kc in range(2):
                        nc.tensor.matmul(
                            vp[:, w_ * 256:(w_ + 1) * 256],
                            lhsT=xnt[:, w_ * 256 + kc * 128: w_ * 256 + (kc + 1) * 128],
                            rhs=wv_s[kc][:],
                            start=(kc == 0), stop=(kc == 1),
                            skip_group_check=True,
                        )
                misc[("qkp", p)] = qkp
                misc[("vp", p)] = vp
                del xnt_p[p]

            # ---- k=9 (pair): qks evac (ACT) + vs evac (DVE) ----
            w = i - 9
            if 0 <= w < nw and w % 2 == 0:
                p = w // 2
                qkp = misc.pop(("qkp", p))
                vp = misc.pop(("vp", p))
                qks = pqks.tile([128, 1024], F16, tag="qks", name="qks")
                nc.scalar.copy(qks[:], qkp[:])
                vs = pvs.tile([128, 512], F16, tag="vs", name="vs")
                nc.vector.tensor_copy(vs[:], vp[:])
                qks_p[p], vs_p[p] = qks, vs

            # ---- k=10 (pair): qz band-scatter DMA (SBUF->SBUF) ----
            w = i - 10
            if 0 <= w < nw and w % 2 == 0:
                p = w // 2
                qks = qks_p[p]
                qz = qz_t[p % 3]
                # Band j: qz[32j:32j+32, j*512:(j+1)*512] <- q half of qks;
                # zeros outside bands were memset once at startup.  The S
                # matmul rhs AP interleaves (j, l) columns at read time.
                for j in range(4):
                    nc.sync.dma_start(
                        qz[32 * j:32 * j + 32, j * 512:(j + 1) * 512],
                        qks[32 * j:32 * j + 32, 0:512])
                misc[("qz", p)] = qz

            # ---- k=11 (window): S matmuls (2x N=512, K=128 padded) ----
            w = i - 11
            if 0 <= w < nw:
                p, w_ = w // 2, w % 2
                qks = qks_p[p]
                qz = misc[("qz", p)]
                sp = psp.tile([128, 1024], F32, tag="sp", name="sp")
                for mh in range(2):
                    rhs = AP(qz[:].tensor,
                             qz[:].offset + mh * 256 + w_ * 128,
                             [[2048, 128], [512, 4], [1, 128]])
                    nc.tensor.matmul(
                        sp[:, mh * 512:(mh + 1) * 512],
                        lhsT=qks[:, 512 + mh * 256 + w_ * 128: 512 + mh * 256 + w_ * 128 + 128],
                        rhs=rhs,
                        start=True, stop=True,
                    )
                misc[("sp", w)] = sp
                if w_ == 1:
                    misc.pop(("qz", p), None)

            # ---- k=12 (window): es = Exp(sp * scale) on ACT ----
            w = i - 12
            if 0 <= w < nw:
                sp = misc.pop(("sp", w))
                es = pes.tile([128, 1024], F16, tag="es", name="es")
                nc.scalar.activation(es[:], sp[:], AF.Exp, scale=inv_sqrt_hd)
                es_w[w] = es

            # ---- k=13 (window): D (4x N=256) + OT (8x) into merged bank ----
            w = i - 13
            if 0 <= w < nw:
                p, w_ = w // 2, w % 2
                es, vs = es_w[w], vs_p[p]
                dop = pd.tile([128, 512], F32, tag="dop", name="dop")
                for j in range(4):
                    rhs = AP(es[:].tensor, es[:].offset + j * 128,
                             [[1024, 128], [512, 2], [1, 128]])
                    nc.tensor.matmul(
                        dop[32 * j:32 * j + 32, 0:256],
                        lhsT=ones_s[:], rhs=rhs,
                        start=True, stop=True, tile_position=(0, 32 * j),
                        skip_group_check=True,
                    )
                for h in range(HEADS):
                    r, j = h // 4, h % 4
                    nc.tensor.matmul(
                        dop[32 * j:32 * j + 32, 256 + r * 128:256 + (r + 1) * 128],
                        lhsT=vs[:, w_ * 256 + 32 * h: w_ * 256 + 32 * h + 32],
                        rhs=es[:, h * 128:(h + 1) * 128],
                        start=(r == 0), stop=(r == 1), tile_position=(0, 32 * j),
                        skip_group_check=True,
                    )
                dop_w[w] = dop
                del es_w[w]

            # ---- k=14 (window): softmax normalize on DVE ----
            w = i - 14
            if 0 <= w < nw:
                p, w_ = w // 2, w % 2
                dop = dop_w.pop(w)
                dbs = pdb.tile([128, 256], F32, tag="dbs", name="dbs")
                nc.vector.reciprocal_approx_fast(out=dbs[:], in_=dop[:, 0:256])
                if w_ == 0:
                    os_p[p] = pos.tile([128, 512], F16, tag="os", name="os")
                os_ = os_p[p]
                nc.vector.scalar_tensor_tensor(
                    out=os_[:, w_ * 256:(w_ + 1) * 256],
                    in0=dop[:, 256:512], scalar=1.0, in1=dbs[:],
                    op0=ALU.mult, op1=ALU.mult,
                )

            # ---- k=15 (pair): out projection (4x N=256) ----
            w = i - 15
            if 0 <= w < nw and w % 2 == 0:
                p = w // 2
                os_ = os_p[p]
                zp = pz.tile([128, 512], F32, tag="zp", name="zp")
                # cols: w'*256 + coh*128 + l
                for coh in range(2):
                    for kc in range(2):
                        rhs = AP(os_[:].tensor, os_[:].offset + kc * 128,
                                 [[512, 128], [256, 2], [1, 128]])
                        out = AP(zp[:].tensor, zp[:].offset + coh * 128,
                                 [[512, 128], [256, 2], [1, 128]])
                        nc.tensor.matmul(
                            out,
                            lhsT=wo_s[kc][:, coh * 128:(coh + 1) * 128],
                            rhs=rhs,
                            start=(kc == 0), stop=(kc == 1),
                            skip_group_check=True,
                        )
                misc[("zp", p)] = zp
                del os_p[p]

            # ---- k=17 (pair): zs evac (ACT) ----
            w = i - 17
            if 0 <= w < nw and w % 2 == 0:
                p = w // 2
                zp = misc.pop(("zp", p))
                zs = pzs.tile([128, 512], F16, tag="zs", name="zs")
                nc.scalar.copy(zs[:], zp[:])
                zs_p[p] = zs
                del qks_p[p], vs_p[p]

            # ---- k=18 (pair): store DMA ----
            w = i - 18
            if 0 <= w < nw and w % 2 == 0:
                p = w // 2
                zs = zs_p.pop(p)
                # zs cols: w'*256 + coh*128 + l ; zt rows (2p+w')*256+coh*128+c
                dst = AP(zt.tensor, (2 * p) * 256 * 128,
                         [[128, 128], [256 * 128, 2], [128 * 128, 2], [1, 128]])
                src = AP(zs[:].tensor, zs[:].offset,
                         [[512, 128], [256, 2], [128, 2], [1, 128]])
                nc.sync.dma_start(dst, src)
    nc.compile()
    return nc


@functools.lru_cache(maxsize=2)
def _get_program(nw: int):
    return _build_program(nw)


def _im2win(x: np.ndarray) -> np.ndarray:
    """[B,T,H,W,C] -> [882,128,256] windows, flat order f = i_n*B + i_b."""
    s = x.strides
    xs = np.lib.stride_tricks.as_strided(
        x,
        shape=(B, T, NHW, PATCH, NHW, PATCH, C),
        strides=(s[0], s[1], STEP * s[2], s[2], STEP * s[3], s[3], s[4]),
    )
    w = xs.transpose(2, 4, 0, 1, 3, 5, 6)  # [iH,iW,b,t,p,q,c]
    return np.ascontiguousarray(w.reshape(NHW * NHW * B, L, C))


def _overlap_add(zwin: np.ndarray, bo: np.ndarray) -> np.ndarray:
    """[882,128,256] window outputs -> [B,T,H,W,C] with count-normalize + bo."""
    th = np.arange(NHW) * STEP
    z = zwin.reshape(B, NHW, NHW, T, PATCH, PATCH, MID)  # [b,iH,iW,t,p,q,c]
    acc = np.zeros((B, T, H, W, MID), np.float32)
    count = np.zeros((H, W), np.float32)
    for p in range(PATCH):
        rid = (th + p)[:, None]
        for q in range(PATCH):
            cid = (th + q)[None, :]
            acc[:, :, rid, cid, :] += z[:, :, :, :, p, q, :].transpose(0, 3, 1, 2, 4)
            count[rid, cid] += 1.0
    out = acc / count[None, None, :, :, None] + bo[None, None, None, None, :]
    return out


LAST_RESULT = None


def kernel(x, ln_g, ln_b, Wq, Wk, Wv, Wo, bo):
    x = np.asarray(x, np.float32)
    ln_g = np.asarray(ln_g, np.float32)
    ln_b = np.asarray(ln_b, np.float32)
    assert np.allclose(ln_b, 0.0), "kernel folds ln_g into weights; ln_b must be 0"
    # Fold LN gamma into the input side of Wq/Wk/Wv.
    wq_t = np.ascontiguousarray((np.asarray(Wq, np.float32) * ln_g).T.astype(np.float16))
    wk_t = np.ascontiguousarray((np.asarray(Wk, np.float32) * ln_g).T.astype(np.float16))
    wv_t = np.ascontiguousarray((np.asarray(Wv, np.float32) * ln_g).T.astype(np.float16))
    wo_t = np.ascontiguousarray(np.asarray(Wo, np.float32).T.astype(np.float16))
    ones1 = np.ones((128, 32), np.float16)
    ident = np.eye(128, dtype=np.float16)

    win = _im2win(x)                              # [882, 128, 256]
    pad = NCORES * NW - NWIN                      # 14
    winp = np.concatenate([win, np.zeros((pad, L, C), np.float32)], 0)
    shards = winp.reshape(NCORES, NW * L, C)

    nc = _get_program(NW)
    trace = bool(int(__import__("os").environ.get("KERNEL_TRACE", "0")))
    in_maps = []
    for i in range(NCORES):
        in_maps.append({
            "xw": np.ascontiguousarray(shards[i]).astype(np.float16),
            "wq": wq_t, "wk": wk_t, "wv": wv_t, "wo": wo_t,
            "ones1": ones1, "ident": ident,
        })
    res = run_bass_kernel_spmd(nc, in_maps, core_ids=list(range(NCORES)),
                               trace=trace)
    global LAST_RESULT
    LAST_RESULT = res
    zts = [np.asarray(res.results[i]["zt"], np.float32).reshape(NW, 2, 128, 128)
           for i in range(NCORES)]
    # zt rows: w*256 + c_out, cols l  ->  Z_w[l, c] = zt[w, :, :, l]
    zall = np.concatenate(zts, 0)                 # [896, 2, 128, 128]
    zwin = zall.reshape(NCORES * NW, MID, L).transpose(0, 2, 1)[:NWIN]
    return _overlap_add(np.ascontiguousarray(zwin), np.asarray(bo, np.float32))
